# revision 9
# baseline (speedup 1.0000x reference)
"""Causal self-attention (B=2, S=2048, D=1024, H=16) on 8 trn2 NeuronCores.

Sharding: core c = b*4 + g handles batch b and head group g (4 heads,
256 features).  Per core:
  - QT/KT computed transposed (feature on partitions, seq on free dim)
    so the QK^T matmul needs no on-chip transposes (x is transposed on
    host, shared by the 4 cores of the batch).
  - logits computed transposed (sk on partitions, sq free): softmax sum
    over sk comes free from an extra ones-column in the V matmul lhsT.
  - causal masking per 128x128 block: fully-masked column ranges are
    memset to zero (never exp'd); the diagonal block gets a triangular
    mask multiply.
  - ctx^T (per head, normalized) is written to a DRAM buffer, AllGather
    over the 4-core group yields the full 1024-feature ctx^T, and each
    core computes its 256-column strip of the output projection.
Host only slices/transposes/concatenates (no arithmetic).
"""

import numpy as np

import concourse.bass as bass
import concourse.mybir as mybir
import concourse.tile as tile
from concourse import bacc
from concourse.bass_utils import run_bass_kernel_spmd

P = 128
B, S, D, H, DH = 2, 2048, 1024, 16, 64
FT = 256          # features per core (4 heads)
NHL = 4           # heads per core
KO = D // P       # 8 k-tiles over the model dim
ST = S // 512     # 4 sq tiles of 512
SB = S // P       # 16 seq blocks of 128
F32 = mybir.dt.float32
F32R = mybir.dt.float32r

TRACE = False
LAST_RESULTS = None
_nc_cache = None


def to_fp32r(a):
    """Round fp32 to the fp32r format (E8M11: low 12 mantissa bits zero),
    round-to-nearest-even, matching the hardware downconv."""
    u = np.ascontiguousarray(a, np.float32).view(np.uint32)
    rb = (u >> 12) & 1
    u = u + 0x7FF + rb
    u = u & np.uint32(0xFFFFF000)
    return u.view(np.float32)


def _mm(nc, out, lhsT, rhs, start, stop):
    nc.tensor.matmul(out, lhsT, rhs, start=start, stop=stop)


def _build_program():
    nc = bacc.Bacc(None, target_bir_lowering=False, debug=False, num_devices=8)

    xT = nc.dram_tensor("xt", [D, S], F32R, kind="ExternalInput").ap()
    wq = nc.dram_tensor("wq", [D, FT], F32R, kind="ExternalInput").ap()
    wk = nc.dram_tensor("wk", [D, FT], F32R, kind="ExternalInput").ap()
    wv = nc.dram_tensor("wv", [D, FT], F32R, kind="ExternalInput").ap()
    wo = nc.dram_tensor("wo", [D, FT], F32R, kind="ExternalInput").ap()
    bq = nc.dram_tensor("bq", [P, 2], F32, kind="ExternalInput").ap()
    bk = nc.dram_tensor("bk", [P, 2], F32, kind="ExternalInput").ap()
    bv = nc.dram_tensor("bv", [1, FT], F32, kind="ExternalInput").ap()
    bo = nc.dram_tensor("bo", [1, FT], F32, kind="ExternalInput").ap()
    tri = nc.dram_tensor("tri", [P, P], F32, kind="ExternalInput").ap()
    out = nc.dram_tensor("out", [S, FT], F32, kind="ExternalOutput").ap()

    with tile.TileContext(nc) as tc:
        with (
            tc.tile_pool(name="const", bufs=1) as constp,
            tc.tile_pool(name="big", bufs=1) as bigp,
            tc.tile_pool(name="qk", bufs=1) as qkp,
            tc.tile_pool(name="attn", bufs=4) as attnp,
            tc.tile_pool(name="small", bufs=3) as smallp,
            tc.tile_pool(name="outp", bufs=3) as outp,
            tc.tile_pool(name="ps_mm", bufs=2, space="PSUM") as ps_mm,
            tc.tile_pool(name="ps_l", bufs=3, space="PSUM") as ps_l,
            tc.tile_pool(name="ps_ctx", bufs=2, space="PSUM") as ps_ctx,
            tc.tile_pool(name="dram", bufs=1, space="DRAM") as dramp,
        ):
            # ---- constants into SBUF
            wq_sb = constp.tile([P, KO, FT], F32R, tag="wq")
            wk_sb = constp.tile([P, KO, FT], F32R, tag="wk")
            wv_sb = constp.tile([P, KO, FT], F32R, tag="wv")
            wo_sb = constp.tile([P, KO, FT], F32R, tag="wo")
            for wsb, wdr in ((wq_sb, wq), (wk_sb, wk), (wv_sb, wv), (wo_sb, wo)):
                nc.sync.dma_start(wsb[:], wdr.rearrange("(ko p) n -> p ko n", p=P))
            bq_sb = constp.tile([P, 2], F32, tag="bq")
            bk_sb = constp.tile([P, 2], F32, tag="bk")
            nc.sync.dma_start(bq_sb[:], bq)
            nc.sync.dma_start(bk_sb[:], bk)
            bv1 = constp.tile([1, FT], F32, tag="bv1")
            bo1 = constp.tile([1, FT], F32, tag="bo1")
            nc.sync.dma_start(bv1[:], bv)
            nc.sync.dma_start(bo1[:], bo)
            bv_sb = constp.tile([P, FT], F32, tag="bvb")
            bo_sb = constp.tile([P, FT], F32, tag="bob")
            nc.gpsimd.partition_broadcast(bv_sb[:], bv1[:], channels=P)
            nc.gpsimd.partition_broadcast(bo_sb[:], bo1[:], channels=P)
            tri_sb = constp.tile([P, P], F32, tag="tri")
            nc.sync.dma_start(tri_sb[:], tri)
            # f32 zero tile: source for writing constants into f32r tiles
            # (memset cannot target float32r)
            z_sb = constp.tile([P, 384], F32, tag="z")
            nc.vector.memset(z_sb[:], 0.0)

            # ---- x^T into SBUF (8 MB), chunked for overlap
            xT_sb = bigp.tile([P, KO, S], F32R, tag="big")
            for ko in range(KO):
                for t in range(ST):
                    nc.sync.dma_start(
                        xT_sb[:, ko, t * 512 : (t + 1) * 512],
                        xT[ko * P : (ko + 1) * P, t * 512 : (t + 1) * 512],
                    )

            # ---- Q^T, K^T projections (feature-major), scale Q by 1/sqrt(dh)
            qt_sb = qkp.tile([P, 2, S], F32R, tag="qt")
            kt_sb = qkp.tile([P, 2, S], F32R, tag="kt")
            for wsb, bsb, scale, dest in (
                (wq_sb, bq_sb, 1.0 / np.sqrt(DH), qt_sb),
                (wk_sb, bk_sb, 1.0, kt_sb),
            ):
                for m in range(2):
                    for t in range(ST):
                        ps = ps_mm.tile([P, 512], F32, tag="psmm")
                        for ko in range(KO):
                            _mm(
                                nc,
                                ps[:, :],
                                wsb[:, ko, m * P : (m + 1) * P],
                                xT_sb[:, ko, t * 512 : (t + 1) * 512],
                                ko == 0,
                                ko == KO - 1,
                            )
                        nc.vector.tensor_scalar(
                            dest[:, m, t * 512 : (t + 1) * 512],
                            ps[:, :],
                            scale,
                            bsb[:, m : m + 1],
                            mybir.AluOpType.mult,
                            mybir.AluOpType.add,
                        )

            # ---- V (natural layout) with an appended ones column per head
            v_sb = qkp.tile([P, SB, NHL * (DH + 1)], F32R, tag="v")
            # ones columns (col 64 of each head's 65-wide strip): (0*0)+1
            v_ones = v_sb[:].rearrange("p s (h c) -> p s h c", c=DH + 1)[:, :, :, DH]
            nc.vector.tensor_scalar(
                v_ones,
                z_sb[:, None, :NHL].to_broadcast((P, SB, NHL)),
                0.0,
                1.0,
                mybir.AluOpType.mult,
                mybir.AluOpType.add,
            )
            for s in range(SB):
                ps = ps_mm.tile([P, 512], F32, tag="psmm")
                for ko in range(KO):
                    _mm(
                        nc,
                        ps[:, :FT],
                        xT_sb[:, ko, s * P : (s + 1) * P],
                        wv_sb[:, ko, :],
                        ko == 0,
                        ko == KO - 1,
                    )
                for h in range(NHL):
                    nc.vector.tensor_add(
                        v_sb[:, s, h * 65 : h * 65 + DH],
                        ps[:, h * DH : (h + 1) * DH],
                        bv_sb[:, h * DH : (h + 1) * DH],
                    )

            # ---- attention, per head / per 512-wide sq tile
            ag_in = dramp.tile([FT, S], F32R)
            for h in range(NHL):
                th, rb = h // 2, (h % 2) * DH
                for t in range(ST):
                    nblk = 4 * t + 4
                    pctx = ps_ctx.tile([P, 512], F32, tag="pctx")
                    for i in range(nblk):
                        pl = ps_l.tile([P, 512], F32, tag="pl")
                        _mm(
                            nc,
                            pl[:, :],
                            kt_sb[rb : rb + DH, th, i * P : (i + 1) * P],
                            qt_sb[rb : rb + DH, th, t * 512 : (t + 1) * 512],
                            True,
                            True,
                        )
                        at = attnp.tile([P, 512], F32R, tag="at")
                        c = i - 4 * t
                        if c < 0:
                            nc.scalar.activation(
                                at[:, :], pl[:, :], mybir.ActivationFunctionType.Exp
                            )
                        else:
                            if c > 0:
                                nc.vector.tensor_scalar_mul(
                                    at[:, : c * P], z_sb[:, : c * P], 1.0
                                )
                            nc.scalar.activation(
                                at[:, c * P :],
                                pl[:, c * P :],
                                mybir.ActivationFunctionType.Exp,
                            )
                            nc.vector.tensor_mul(
                                at[:, c * P : (c + 1) * P],
                                at[:, c * P : (c + 1) * P],
                                tri_sb[:, :],
                            )
                        _mm(
                            nc,
                            pctx[0 : DH + 1, :],
                            v_sb[:, i, h * 65 : (h + 1) * 65],
                            at[:, :],
                            i == 0,
                            i == nblk - 1,
                        )
                    # normalize: row DH of pctx holds the softmax denominator
                    rs = smallp.tile([1, 512], F32, tag="rs")
                    nc.vector.reciprocal(rs[:], pctx[DH : DH + 1, :])
                    bc = smallp.tile([DH, 512], F32, tag="bc")
                    nc.gpsimd.partition_broadcast(bc[:], rs[:], channels=DH)
                    cn = smallp.tile([DH, 512], F32R, tag="cn")
                    nc.vector.tensor_mul(cn[:], pctx[0:DH, :], bc[:])
                    nc.sync.dma_start(
                        ag_in[h * DH : (h + 1) * DH, t * 512 : (t + 1) * 512], cn[:]
                    )

            # ---- AllGather ctx^T across the 4-core group
            ag_out = dramp.tile([D, S], F32R)
            nc.gpsimd.collective_compute(
                "AllGather",
                mybir.AluOpType.bypass,
                replica_groups=[[0, 1, 2, 3], [4, 5, 6, 7]],
                ins=[ag_in.opt()],
                outs=[ag_out.opt()],
            )

            # ---- output projection: out[:, my 256 cols] = ctx @ Wo[:, cols] + bo
            lh = bigp.tile([P, KO, S], F32R, tag="big")
            for ko in range(KO):
                nc.sync.dma_start(lh[:, ko, :], ag_out[ko * P : (ko + 1) * P, :])
            for m in range(SB):
                ps = ps_mm.tile([P, 512], F32, tag="psmm")
                for ko in range(KO):
                    _mm(
                        nc,
                        ps[:, :FT],
                        lh[:, ko, m * P : (m + 1) * P],
                        wo_sb[:, ko, :],
                        ko == 0,
                        ko == KO - 1,
                    )
                ob = outp.tile([P, FT], F32, tag="ob")
                nc.vector.tensor_add(ob[:], ps[:, :FT], bo_sb[:])
                nc.sync.dma_start(out[m * P : (m + 1) * P, :], ob[:])

    nc.finalize()
    return nc


def kernel(x, Wq, bq, Wk, bk, Wv, bv, Wo, bo):
    global _nc_cache, LAST_RESULTS
    if _nc_cache is None:
        _nc_cache = _build_program()

    x = np.asarray(x, np.float32)
    Wq, Wk, Wv, Wo = (np.asarray(a, np.float32) for a in (Wq, Wk, Wv, Wo))
    bq, bk, bv, bo = (np.asarray(a, np.float32) for a in (bq, bk, bv, bo))
    tri = np.triu(np.ones((P, P), np.float32))

    xts = [to_fp32r(np.ascontiguousarray(x[b].T)) for b in range(B)]
    in_maps = []
    for c in range(8):
        b, g = divmod(c, 4)
        sl = slice(g * FT, (g + 1) * FT)
        in_maps.append(
            {
                "xt": xts[b],
                "wq": to_fp32r(Wq[:, sl]),
                "wk": to_fp32r(Wk[:, sl]),
                "wv": to_fp32r(Wv[:, sl]),
                "wo": to_fp32r(Wo[:, sl]),
                "bq": np.ascontiguousarray(bq[sl].reshape(2, P).T),
                "bk": np.ascontiguousarray(bk[sl].reshape(2, P).T),
                "bv": bv[sl].reshape(1, FT).copy(),
                "bo": bo[sl].reshape(1, FT).copy(),
                "tri": tri,
            }
        )

    LAST_RESULTS = run_bass_kernel_spmd(
        _nc_cache, in_maps, list(range(8)), trace=TRACE
    )
    res = LAST_RESULTS.results
    outp = np.empty((B, S, D), np.float32)
    for c in range(8):
        b, g = divmod(c, 4)
        outp[b, :, g * FT : (g + 1) * FT] = res[c]["out"]
    return outp


# revision 13
# speedup vs baseline: 1.1784x; 1.1784x over previous
"""Causal self-attention (B=2, S=2048, D=1024, H=16) on 8 trn2 NeuronCores.

Sharding: core c = b*4 + g handles batch b and head group g (4 heads,
256 features).  Per core:
  - QT/KT computed transposed (feature on partitions, seq on free dim)
    so the QK^T matmul needs no on-chip transposes (x is transposed on
    host, shared by the 4 cores of the batch).
  - logits computed transposed (sk on partitions, sq free) in 1024-wide
    tiles: softmax sum over sk comes free from an extra ones-column in
    the V matmul lhsT; exp runs as one wide ACT op per sk-block.
  - attention operands (q/k/v/attn weights) are bf16 on the PE (fp32
    PSUM accumulation); projections and the output GEMM use fp32r
    (fp32 rounded to 11 mantissa bits, full-rate on the PE).
  - causal masking per 128x128 block: fully-masked column ranges are
    memset to zero (never exp'd); the diagonal block gets a triangular
    mask multiply.
  - ctx^T (per head, normalized) is AllGather'd per head over the
    4-core group (overlapping the next head's compute); each core then
    computes its 256-column strip of the output projection.
Host only slices/transposes/permutes/concatenates (no arithmetic).
"""

import numpy as np

import concourse.bass as bass
import concourse.mybir as mybir
import concourse.tile as tile
from concourse import bacc
from concourse.bass_utils import run_bass_kernel_spmd

P = 128
B, S, D, H, DH = 2, 2048, 1024, 16, 64
FT = 256          # features per core (4 heads)
NHL = 4           # heads per core
KO = D // P       # 8 k-tiles over the model dim
ST = S // 512     # 4 sq tiles of 512
SB = S // P       # 16 seq blocks of 128
F32 = mybir.dt.float32
F32R = mybir.dt.float32r
BF16 = mybir.dt.bfloat16

TRACE = False
USE_DIVIDE = False
LAST_RESULTS = None
_nc_cache = None

try:
    import ml_dtypes
    _BF16_NP = np.dtype(ml_dtypes.bfloat16)
except ImportError:  # pragma: no cover
    _BF16_NP = np.float32


def to_fp32r(a):
    """Round fp32 to the fp32r format (E8M11: low 12 mantissa bits zero),
    round-to-nearest-even, matching the hardware downconv."""
    u = np.ascontiguousarray(a, np.float32).view(np.uint32)
    rb = (u >> 12) & 1
    u = u + 0x7FF + rb
    u = u & np.uint32(0xFFFFF000)
    return u.view(np.float32)


def _build_program():
    nc = bacc.Bacc(None, target_bir_lowering=False, debug=False, num_devices=8)

    xT = nc.dram_tensor("xt", [D, S], F32R, kind="ExternalInput").ap()
    wq = nc.dram_tensor("wq", [D, FT], F32R, kind="ExternalInput").ap()
    wk = nc.dram_tensor("wk", [D, FT], F32R, kind="ExternalInput").ap()
    wv = nc.dram_tensor("wv", [D, FT], F32R, kind="ExternalInput").ap()
    wo = nc.dram_tensor("wo", [D, FT], F32R, kind="ExternalInput").ap()
    bq = nc.dram_tensor("bq", [P, 2], F32, kind="ExternalInput").ap()
    bk = nc.dram_tensor("bk", [P, 2], F32, kind="ExternalInput").ap()
    bv = nc.dram_tensor("bv", [1, FT], F32, kind="ExternalInput").ap()
    bo = nc.dram_tensor("bo", [1, FT], F32, kind="ExternalInput").ap()
    tri = nc.dram_tensor("tri", [P, P], BF16, kind="ExternalInput").ap()
    out = nc.dram_tensor("out", [S, FT], F32, kind="ExternalOutput").ap()

    with tile.TileContext(nc) as tc:
        with (
            tc.tile_pool(name="const", bufs=1) as constp,
            tc.tile_pool(name="big", bufs=1) as bigp,
            tc.tile_pool(name="qk", bufs=1) as qkp,
            tc.tile_pool(name="attn", bufs=4) as attnp,
            tc.tile_pool(name="small", bufs=3) as smallp,
            tc.tile_pool(name="outp", bufs=3) as outp,
            tc.tile_pool(name="ps_mm", bufs=2, space="PSUM") as ps_mm,
            tc.tile_pool(name="ps_l", bufs=2, space="PSUM") as ps_l,
            tc.tile_pool(name="ps_ctx", bufs=2, space="PSUM") as ps_ctx,
            tc.tile_pool(name="dram", bufs=1, space="DRAM") as dramp,
        ):
            # ---- constants into SBUF
            wq_sb = constp.tile([P, KO, FT], F32R, tag="wq")
            wk_sb = constp.tile([P, KO, FT], F32R, tag="wk")
            wv_sb = constp.tile([P, KO, FT], F32R, tag="wv")
            wo_sb = constp.tile([P, KO, FT], F32R, tag="wo")
            for wsb, wdr in ((wq_sb, wq), (wk_sb, wk), (wv_sb, wv), (wo_sb, wo)):
                nc.sync.dma_start(wsb[:], wdr.rearrange("(ko p) n -> p ko n", p=P))
            bq_sb = constp.tile([P, 2], F32, tag="bq")
            bk_sb = constp.tile([P, 2], F32, tag="bk")
            nc.sync.dma_start(bq_sb[:], bq)
            nc.sync.dma_start(bk_sb[:], bk)
            bv_sb = constp.tile([P, FT], F32, tag="bvb")
            bo_sb = constp.tile([P, FT], F32, tag="bob")
            nc.sync.dma_start(bv_sb[:], bv.partition_broadcast(P))
            nc.sync.dma_start(bo_sb[:], bo.partition_broadcast(P))
            tri_sb = constp.tile([P, P], BF16, tag="tri")
            nc.sync.dma_start(tri_sb[:], tri)
            # f32 zero tile: source for writing constants into f32r/bf16 tiles
            z_sb = constp.tile([P, 8], F32, tag="z")
            nc.vector.memset(z_sb[:], 0.0)

            # ---- x^T into SBUF (8 MB) on the gpsimd queue, one DMA per k-tile
            xT_sb = bigp.tile([P, KO, S], F32R, tag="big")
            for ko in range(KO):
                nc.gpsimd.dma_start(xT_sb[:, ko, :], xT[ko * P : (ko + 1) * P, :])

            # ---- Q^T, K^T projections (feature-major, bf16 out), Q scaled
            qt_sb = qkp.tile([P, 2, S], BF16, tag="qt")
            kt_sb = qkp.tile([P, 2, S], BF16, tag="kt")
            for wsb, bsb, scale, dest in (
                (wq_sb, bq_sb, 1.0 / np.sqrt(DH), qt_sb),
                (wk_sb, bk_sb, 1.0, kt_sb),
            ):
                for m in range(2):
                    for t in range(ST):
                        ps = ps_mm.tile([P, 512], F32, tag="psmm")
                        for ko in range(KO):
                            nc.tensor.matmul(
                                ps[:, :],
                                wsb[:, ko, m * P : (m + 1) * P],
                                xT_sb[:, ko, t * 512 : (t + 1) * 512],
                                start=(ko == 0),
                                stop=(ko == KO - 1),
                            )
                        nc.vector.tensor_scalar(
                            dest[:, m, t * 512 : (t + 1) * 512],
                            ps[:, :],
                            scale,
                            bsb[:, m : m + 1],
                            mybir.AluOpType.mult,
                            mybir.AluOpType.add,
                        )

            # ---- V (natural layout, bf16) with an appended ones column per head
            v_sb = qkp.tile([P, SB, NHL * (DH + 1)], BF16, tag="v")
            v_ones = v_sb[:].rearrange("p s (h c) -> p s h c", c=DH + 1)[:, :, :, DH]
            nc.vector.tensor_scalar(
                v_ones,
                z_sb[:, None, :NHL].to_broadcast((P, SB, NHL)),
                0.0,
                1.0,
                mybir.AluOpType.mult,
                mybir.AluOpType.add,
            )
            for s in range(SB):
                ps = ps_mm.tile([P, 512], F32, tag="psmm")
                for ko in range(KO):
                    nc.tensor.matmul(
                        ps[:, :FT],
                        xT_sb[:, ko, s * P : (s + 1) * P],
                        wv_sb[:, ko, :],
                        start=(ko == 0),
                        stop=(ko == KO - 1),
                    )
                for h in range(NHL):
                    nc.vector.tensor_add(
                        v_sb[:, s, h * 65 : h * 65 + DH],
                        ps[:, h * DH : (h + 1) * DH],
                        bv_sb[:, h * DH : (h + 1) * DH],
                    )

            # ---- attention: per head, 1024-wide sq tiles; AllGather per head
            ag_in = [
                dramp.tile([DH, S], F32R, name=f"ag_in{h}") for h in range(NHL)
            ]
            ag_out = [
                dramp.tile([4 * DH, S], F32R, name=f"ag_out{h}") for h in range(NHL)
            ]
            lh = bigp.tile([P, KO, S], F32R, tag="big")  # Wo lhsT, reuses xT slot

            for h in range(NHL):
                th, rb = h // 2, (h % 2) * DH
                for t2 in range(2):
                    sq0 = t2 * 1024
                    nblk = 8 * t2 + 8   # sk blocks 0 .. 8*t2+7
                    pctx = [
                        ps_ctx.tile([P, 512], F32, tag="pctx", name=f"pctx{h}_{t2}_{q}")
                        for q in range(2)
                    ]
                    for i in range(nblk):
                        v0 = i <= 8 * t2 + 3     # half0 (sq0..sq0+511) has live cols
                        pl = ps_l.tile([P, 1024], F32, tag="pl")
                        if v0:
                            nc.tensor.matmul(
                                pl[:, 0:512],
                                kt_sb[rb : rb + DH, th, i * P : (i + 1) * P],
                                qt_sb[rb : rb + DH, th, sq0 : sq0 + 512],
                                start=True,
                                stop=True,
                            )
                        nc.tensor.matmul(
                            pl[:, 512:1024],
                            kt_sb[rb : rb + DH, th, i * P : (i + 1) * P],
                            qt_sb[rb : rb + DH, th, sq0 + 512 : sq0 + 1024],
                            start=True,
                            stop=True,
                        )
                        at = attnp.tile([P, 1024], BF16, tag="at")
                        if v0:
                            c = i - 8 * t2       # diag block index within half0
                            if c >= 0:
                                if c > 0:
                                    nc.vector.memset(at[:, : c * P], 0.0)
                                nc.scalar.activation(
                                    at[:, c * P :],
                                    pl[:, c * P :],
                                    mybir.ActivationFunctionType.Exp,
                                )
                                nc.vector.tensor_mul(
                                    at[:, c * P : (c + 1) * P],
                                    at[:, c * P : (c + 1) * P],
                                    tri_sb[:, :],
                                )
                            else:
                                nc.scalar.activation(
                                    at[:, :], pl[:, :],
                                    mybir.ActivationFunctionType.Exp,
                                )
                        else:
                            c = i - (8 * t2 + 4)  # diag block within half1
                            base = 512 + c * P
                            if c > 0:
                                nc.vector.memset(at[:, 512 : base], 0.0)
                            nc.scalar.activation(
                                at[:, base:],
                                pl[:, base:],
                                mybir.ActivationFunctionType.Exp,
                            )
                            nc.vector.tensor_mul(
                                at[:, base : base + P],
                                at[:, base : base + P],
                                tri_sb[:, :],
                            )
                        if v0:
                            nc.tensor.matmul(
                                pctx[0][0 : DH + 1, :],
                                v_sb[:, i, h * 65 : (h + 1) * 65],
                                at[:, 0:512],
                                start=(i == 0),
                                stop=(i == 8 * t2 + 3),
                            )
                        nc.tensor.matmul(
                            pctx[1][0 : DH + 1, :],
                            v_sb[:, i, h * 65 : (h + 1) * 65],
                            at[:, 512:1024],
                            start=(i == 0),
                            stop=(i == nblk - 1),
                        )
                    # normalize the two finished 512-quarters
                    for hh in range(2):
                        tq = 2 * t2 + hh
                        cn = smallp.tile([DH, 512], F32R, tag="cn")
                        bc = smallp.tile([DH, 512], F32, tag="bc")
                        if USE_DIVIDE:
                            rsum = smallp.tile([1, 512], F32, tag="rs")
                            nc.vector.tensor_copy(
                                rsum[:], pctx[hh][DH : DH + 1, :]
                            )
                            nc.gpsimd.partition_broadcast(
                                bc[:], rsum[:], channels=DH
                            )
                            nc.vector.tensor_tensor(
                                cn[:], pctx[hh][0:DH, :], bc[:],
                                mybir.AluOpType.divide,
                            )
                        else:
                            rs = smallp.tile([1, 512], F32, tag="rs")
                            nc.vector.reciprocal(rs[:], pctx[hh][DH : DH + 1, :])
                            nc.gpsimd.partition_broadcast(
                                bc[:], rs[:], channels=DH
                            )
                            nc.vector.tensor_mul(cn[:], pctx[hh][0:DH, :], bc[:])
                        nc.sync.dma_start(
                            ag_in[h][:, tq * 512 : (tq + 1) * 512], cn[:]
                        )
                # per-head AllGather (overlaps the next head's compute)
                nc.gpsimd.collective_compute(
                    "AllGather",
                    mybir.AluOpType.bypass,
                    replica_groups=[[0, 1, 2, 3], [4, 5, 6, 7]],
                    ins=[ag_in[h].opt()],
                    outs=[ag_out[h].opt()],
                )
                for kk in range(2):
                    nc.gpsimd.dma_start(
                        lh[:, 2 * h + kk, :],
                        ag_out[h][kk * P : (kk + 1) * P, :],
                    )

            # ---- output projection: out[:, my 256 cols] = ctx @ Wo_perm + bo
            for m in range(SB):
                ps = ps_mm.tile([P, 512], F32, tag="psmm")
                for ko in range(KO):
                    nc.tensor.matmul(
                        ps[:, :FT],
                        lh[:, ko, m * P : (m + 1) * P],
                        wo_sb[:, ko, :],
                        start=(ko == 0),
                        stop=(ko == KO - 1),
                    )
                ob = outp.tile([P, FT], F32, tag="ob")
                nc.vector.tensor_add(ob[:], ps[:, :FT], bo_sb[:])
                nc.sync.dma_start(out[m * P : (m + 1) * P, :], ob[:])

    nc.finalize()
    return nc


def kernel(x, Wq, bq, Wk, bk, Wv, bv, Wo, bo):
    global _nc_cache, LAST_RESULTS
    if _nc_cache is None:
        _nc_cache = _build_program()

    x = np.asarray(x, np.float32)
    Wq, Wk, Wv, Wo = (np.asarray(a, np.float32) for a in (Wq, Wk, Wv, Wo))
    bq, bk, bv, bo = (np.asarray(a, np.float32) for a in (bq, bk, bv, bo))
    tri = np.triu(np.ones((P, P), np.float32)).astype(_BF16_NP)

    # Wo row permutation: the gathered ctx^T k-tiles arrive ordered
    # (head h, group g, row r) -> global feature g*256 + h*64 + r
    perm = np.array(
        [
            j * FT + h * DH + r
            for h in range(NHL)
            for j in range(4)
            for r in range(DH)
        ]
    )

    xts = [to_fp32r(np.ascontiguousarray(x[b].T)) for b in range(B)]
    in_maps = []
    for c in range(8):
        b, g = divmod(c, 4)
        sl = slice(g * FT, (g + 1) * FT)
        in_maps.append(
            {
                "xt": xts[b],
                "wq": to_fp32r(Wq[:, sl]),
                "wk": to_fp32r(Wk[:, sl]),
                "wv": to_fp32r(Wv[:, sl]),
                "wo": to_fp32r(Wo[perm][:, sl]),
                "bq": np.ascontiguousarray(bq[sl].reshape(2, P).T),
                "bk": np.ascontiguousarray(bk[sl].reshape(2, P).T),
                "bv": bv[sl].reshape(1, FT).copy(),
                "bo": bo[sl].reshape(1, FT).copy(),
                "tri": tri,
            }
        )

    LAST_RESULTS = run_bass_kernel_spmd(
        _nc_cache, in_maps, list(range(8)), trace=TRACE
    )
    res = LAST_RESULTS.results
    outp = np.empty((B, S, D), np.float32)
    for c in range(8):
        b, g = divmod(c, 4)
        outp[b, :, g * FT : (g + 1) * FT] = res[c]["out"]
    return outp


# revision 16
# speedup vs baseline: 1.1882x; 1.0083x over previous
"""Causal self-attention (B=2, S=2048, D=1024, H=16) on 8 trn2 NeuronCores.

Sharding: core c = b*4 + g handles batch b and head group g (4 heads,
256 features).  Per core:
  - QT/KT computed transposed (feature on partitions, seq on free dim)
    so the QK^T matmul needs no on-chip transposes (x is transposed on
    host, shared by the 4 cores of the batch).
  - logits computed transposed (sk on partitions, sq free) in 1024-wide
    tiles: softmax sum over sk comes free from an extra ones-column in
    the V matmul lhsT; exp runs as one wide ACT op per sk-block.
  - attention operands (q/k/v/attn weights) are bf16 on the PE (fp32
    PSUM accumulation); projections and the output GEMM use fp32r
    (fp32 rounded to 11 mantissa bits, full-rate on the PE).
  - causal masking per 128x128 block: fully-masked column ranges are
    memset to zero (never exp'd); the diagonal block gets a triangular
    mask multiply.
  - ctx^T (per head, normalized) is AllGather'd per head over the
    4-core group (overlapping the next head's compute); each core then
    computes its 256-column strip of the output projection.
Host only slices/transposes/permutes/concatenates (no arithmetic).
"""

import numpy as np

import concourse.bass as bass
import concourse.mybir as mybir
import concourse.tile as tile
from concourse import bacc
from concourse.bass_utils import run_bass_kernel_spmd

P = 128
B, S, D, H, DH = 2, 2048, 1024, 16, 64
FT = 256          # features per core (4 heads)
NHL = 4           # heads per core
KO = D // P       # 8 k-tiles over the model dim
ST = S // 512     # 4 sq tiles of 512
SB = S // P       # 16 seq blocks of 128
F32 = mybir.dt.float32
F32R = mybir.dt.float32r
BF16 = mybir.dt.bfloat16

TRACE = False
USE_DIVIDE = False
LAST_RESULTS = None
_nc_cache = None

try:
    import ml_dtypes
    _BF16_NP = np.dtype(ml_dtypes.bfloat16)
except ImportError:  # pragma: no cover
    _BF16_NP = np.float32


def to_fp32r(a):
    """Round fp32 to the fp32r format (E8M11: low 12 mantissa bits zero),
    round-to-nearest-even, matching the hardware downconv."""
    u = np.ascontiguousarray(a, np.float32).view(np.uint32)
    rb = (u >> 12) & 1
    u = u + 0x7FF + rb
    u = u & np.uint32(0xFFFFF000)
    return u.view(np.float32)


def _build_program():
    nc = bacc.Bacc(None, target_bir_lowering=False, debug=False, num_devices=8)

    xT = nc.dram_tensor("xt", [D, S], F32R, kind="ExternalInput").ap()
    wq = nc.dram_tensor("wq", [D, FT], F32R, kind="ExternalInput").ap()
    wk = nc.dram_tensor("wk", [D, FT], F32R, kind="ExternalInput").ap()
    wv = nc.dram_tensor("wv", [D, FT], F32R, kind="ExternalInput").ap()
    wo = nc.dram_tensor("wo", [D, FT], F32R, kind="ExternalInput").ap()
    bq = nc.dram_tensor("bq", [P, 2], F32, kind="ExternalInput").ap()
    bk = nc.dram_tensor("bk", [P, 2], F32, kind="ExternalInput").ap()
    bv = nc.dram_tensor("bv", [1, FT], F32, kind="ExternalInput").ap()
    bo = nc.dram_tensor("bo", [1, FT], F32, kind="ExternalInput").ap()
    tri = nc.dram_tensor("tri", [P, P], BF16, kind="ExternalInput").ap()
    out = nc.dram_tensor("out", [S, FT], F32, kind="ExternalOutput").ap()

    with tile.TileContext(nc) as tc:
        with (
            tc.tile_pool(name="const", bufs=1) as constp,
            tc.tile_pool(name="big", bufs=1) as bigp,
            tc.tile_pool(name="qk", bufs=1) as qkp,
            tc.tile_pool(name="attn", bufs=6) as attnp,
            tc.tile_pool(name="small", bufs=6) as smallp,
            tc.tile_pool(name="outp", bufs=3) as outp,
            tc.tile_pool(name="ps_mm", bufs=2, space="PSUM") as ps_mm,
            tc.tile_pool(name="ps_l", bufs=2, space="PSUM") as ps_l,
            tc.tile_pool(name="ps_ctx", bufs=2, space="PSUM") as ps_ctx,
            tc.tile_pool(name="dram", bufs=1, space="DRAM") as dramp,
        ):
            # ---- constants into SBUF
            wq_sb = constp.tile([P, KO, FT], F32R, tag="wq")
            wk_sb = constp.tile([P, KO, FT], F32R, tag="wk")
            wv_sb = constp.tile([P, KO, FT], F32R, tag="wv")
            wo_sb = constp.tile([P, KO, FT], F32R, tag="wo")
            for wsb, wdr in ((wq_sb, wq), (wk_sb, wk), (wv_sb, wv), (wo_sb, wo)):
                nc.sync.dma_start(wsb[:], wdr.rearrange("(ko p) n -> p ko n", p=P))
            bq_sb = constp.tile([P, 2], F32, tag="bq")
            bk_sb = constp.tile([P, 2], F32, tag="bk")
            nc.sync.dma_start(bq_sb[:], bq)
            nc.sync.dma_start(bk_sb[:], bk)
            bv_sb = constp.tile([P, FT], F32, tag="bvb")
            bo_sb = constp.tile([P, FT], F32, tag="bob")
            nc.sync.dma_start(bv_sb[:], bv.partition_broadcast(P))
            nc.sync.dma_start(bo_sb[:], bo.partition_broadcast(P))
            tri_sb = constp.tile([P, P], BF16, tag="tri")
            nc.sync.dma_start(tri_sb[:], tri)
            # f32 zero tile: source for writing constants into f32r/bf16 tiles
            z_sb = constp.tile([P, 8], F32, tag="z")
            nc.vector.memset(z_sb[:], 0.0)

            # ---- x^T into SBUF (8 MB) on the gpsimd queue, one DMA per k-tile
            xT_sb = bigp.tile([P, KO, S], F32R, tag="big")
            for ko in range(KO):
                for half in range(2):
                    eng = nc.gpsimd if (ko + half) % 2 == 0 else nc.sync
                    eng.dma_start(
                        xT_sb[:, ko, half * 1024 : (half + 1) * 1024],
                        xT[ko * P : (ko + 1) * P, half * 1024 : (half + 1) * 1024],
                    )

            # ---- Q^T, K^T projections (feature-major, bf16 out), Q scaled
            qt_sb = qkp.tile([P, 2, S], BF16, tag="qt")
            kt_sb = qkp.tile([P, 2, S], BF16, tag="kt")
            for wsb, bsb, scale, dest in (
                (wq_sb, bq_sb, 1.0 / np.sqrt(DH), qt_sb),
                (wk_sb, bk_sb, 1.0, kt_sb),
            ):
                for m in range(2):
                    for t in range(ST):
                        ps = ps_mm.tile([P, 512], F32, tag="psmm")
                        for ko in range(KO):
                            nc.tensor.matmul(
                                ps[:, :],
                                wsb[:, ko, m * P : (m + 1) * P],
                                xT_sb[:, ko, t * 512 : (t + 1) * 512],
                                start=(ko == 0),
                                stop=(ko == KO - 1),
                            )
                        nc.vector.tensor_scalar(
                            dest[:, m, t * 512 : (t + 1) * 512],
                            ps[:, :],
                            scale,
                            bsb[:, m : m + 1],
                            mybir.AluOpType.mult,
                            mybir.AluOpType.add,
                        )

            # ---- V (natural layout, bf16) with an appended ones column per head
            v_sb = qkp.tile([P, SB, NHL * (DH + 1)], BF16, tag="v")
            v_ones = v_sb[:].rearrange("p s (h c) -> p s h c", c=DH + 1)[:, :, :, DH]
            nc.vector.tensor_scalar(
                v_ones,
                z_sb[:, None, :NHL].to_broadcast((P, SB, NHL)),
                0.0,
                1.0,
                mybir.AluOpType.mult,
                mybir.AluOpType.add,
            )
            for s in range(SB):
                ps = ps_mm.tile([P, 512], F32, tag="psmm")
                for ko in range(KO):
                    nc.tensor.matmul(
                        ps[:, :FT],
                        xT_sb[:, ko, s * P : (s + 1) * P],
                        wv_sb[:, ko, :],
                        start=(ko == 0),
                        stop=(ko == KO - 1),
                    )
                for h in range(NHL):
                    nc.vector.tensor_add(
                        v_sb[:, s, h * 65 : h * 65 + DH],
                        ps[:, h * DH : (h + 1) * DH],
                        bv_sb[:, h * DH : (h + 1) * DH],
                    )

            # ---- attention: per head, 1024-wide sq tiles; AllGather per head
            ag_in = [
                dramp.tile([DH, S], F32R, name=f"ag_in{h}") for h in range(NHL)
            ]
            ag_out = [
                dramp.tile([4 * DH, S], F32R, name=f"ag_out{h}") for h in range(NHL)
            ]
            lh = bigp.tile([P, KO, S], F32R, tag="big")  # Wo lhsT, reuses xT slot

            for h in range(NHL):
                th, rb = h // 2, (h % 2) * DH
                for t2 in range(2):
                    sq0 = t2 * 1024
                    nblk = 8 * t2 + 8   # sk blocks 0 .. 8*t2+7
                    pctx = [
                        ps_ctx.tile([P, 512], F32, tag="pctx", name=f"pctx{h}_{t2}_{q}")
                        for q in range(2)
                    ]
                    for i in range(nblk):
                        v0 = i <= 8 * t2 + 3     # half0 (sq0..sq0+511) has live cols
                        pl = ps_l.tile([P, 1024], F32, tag="pl")
                        if v0:
                            nc.tensor.matmul(
                                pl[:, 0:512],
                                kt_sb[rb : rb + DH, th, i * P : (i + 1) * P],
                                qt_sb[rb : rb + DH, th, sq0 : sq0 + 512],
                                start=True,
                                stop=True,
                            )
                        nc.tensor.matmul(
                            pl[:, 512:1024],
                            kt_sb[rb : rb + DH, th, i * P : (i + 1) * P],
                            qt_sb[rb : rb + DH, th, sq0 + 512 : sq0 + 1024],
                            start=True,
                            stop=True,
                        )
                        at = attnp.tile([P, 1024], BF16, tag="at")
                        if v0:
                            c = i - 8 * t2       # diag block index within half0
                            if c >= 0:
                                if c > 0:
                                    nc.vector.memset(at[:, : c * P], 0.0)
                                nc.scalar.activation(
                                    at[:, c * P :],
                                    pl[:, c * P :],
                                    mybir.ActivationFunctionType.Exp,
                                )
                                nc.vector.tensor_mul(
                                    at[:, c * P : (c + 1) * P],
                                    at[:, c * P : (c + 1) * P],
                                    tri_sb[:, :],
                                )
                            else:
                                nc.scalar.activation(
                                    at[:, :], pl[:, :],
                                    mybir.ActivationFunctionType.Exp,
                                )
                        else:
                            c = i - (8 * t2 + 4)  # diag block within half1
                            base = 512 + c * P
                            if c > 0:
                                nc.vector.memset(at[:, 512 : base], 0.0)
                            nc.scalar.activation(
                                at[:, base:],
                                pl[:, base:],
                                mybir.ActivationFunctionType.Exp,
                            )
                            nc.vector.tensor_mul(
                                at[:, base : base + P],
                                at[:, base : base + P],
                                tri_sb[:, :],
                            )
                        if v0:
                            nc.tensor.matmul(
                                pctx[0][0 : DH + 1, :],
                                v_sb[:, i, h * 65 : (h + 1) * 65],
                                at[:, 0:512],
                                start=(i == 0),
                                stop=(i == 8 * t2 + 3),
                            )
                        nc.tensor.matmul(
                            pctx[1][0 : DH + 1, :],
                            v_sb[:, i, h * 65 : (h + 1) * 65],
                            at[:, 512:1024],
                            start=(i == 0),
                            stop=(i == nblk - 1),
                        )
                    # normalize the two finished 512-quarters
                    for hh in range(2):
                        tq = 2 * t2 + hh
                        cn = smallp.tile([DH, 512], F32R, tag="cn")
                        bc = smallp.tile([DH, 512], F32, tag="bc")
                        if USE_DIVIDE:
                            rsum = smallp.tile([1, 512], F32, tag="rs")
                            nc.vector.tensor_copy(
                                rsum[:], pctx[hh][DH : DH + 1, :]
                            )
                            nc.gpsimd.partition_broadcast(
                                bc[:], rsum[:], channels=DH
                            )
                            nc.vector.tensor_tensor(
                                cn[:], pctx[hh][0:DH, :], bc[:],
                                mybir.AluOpType.divide,
                            )
                        else:
                            rs = smallp.tile([1, 512], F32, tag="rs")
                            nc.vector.reciprocal(rs[:], pctx[hh][DH : DH + 1, :])
                            nc.gpsimd.partition_broadcast(
                                bc[:], rs[:], channels=DH
                            )
                            nc.vector.tensor_mul(cn[:], pctx[hh][0:DH, :], bc[:])
                        nc.sync.dma_start(
                            ag_in[h][:, tq * 512 : (tq + 1) * 512], cn[:]
                        )
                # per-head AllGather (overlaps the next head's compute)
                nc.gpsimd.collective_compute(
                    "AllGather",
                    mybir.AluOpType.bypass,
                    replica_groups=[[0, 1, 2, 3], [4, 5, 6, 7]],
                    ins=[ag_in[h].opt()],
                    outs=[ag_out[h].opt()],
                )

            # gathered ctx^T k-tiles -> SBUF (issued after attention so the
            # AG-completion waits don't block any queue mid-attention)
            for h in range(NHL):
                for kk in range(2):
                    nc.sync.dma_start(
                        lh[:, 2 * h + kk, :],
                        ag_out[h][kk * P : (kk + 1) * P, :],
                    )

            # ---- output projection: out[:, my 256 cols] = ctx @ Wo_perm + bo
            for m in range(SB):
                ps = ps_mm.tile([P, 512], F32, tag="psmm")
                for ko in range(KO):
                    nc.tensor.matmul(
                        ps[:, :FT],
                        lh[:, ko, m * P : (m + 1) * P],
                        wo_sb[:, ko, :],
                        start=(ko == 0),
                        stop=(ko == KO - 1),
                    )
                ob = outp.tile([P, FT], F32, tag="ob")
                nc.vector.tensor_add(ob[:], ps[:, :FT], bo_sb[:])
                nc.sync.dma_start(out[m * P : (m + 1) * P, :], ob[:])

    nc.finalize()
    return nc


def kernel(x, Wq, bq, Wk, bk, Wv, bv, Wo, bo):
    global _nc_cache, LAST_RESULTS
    if _nc_cache is None:
        _nc_cache = _build_program()

    x = np.asarray(x, np.float32)
    Wq, Wk, Wv, Wo = (np.asarray(a, np.float32) for a in (Wq, Wk, Wv, Wo))
    bq, bk, bv, bo = (np.asarray(a, np.float32) for a in (bq, bk, bv, bo))
    tri = np.triu(np.ones((P, P), np.float32)).astype(_BF16_NP)

    # Wo row permutation: the gathered ctx^T k-tiles arrive ordered
    # (head h, group g, row r) -> global feature g*256 + h*64 + r
    perm = np.array(
        [
            j * FT + h * DH + r
            for h in range(NHL)
            for j in range(4)
            for r in range(DH)
        ]
    )

    xts = [to_fp32r(np.ascontiguousarray(x[b].T)) for b in range(B)]
    in_maps = []
    for c in range(8):
        b, g = divmod(c, 4)
        sl = slice(g * FT, (g + 1) * FT)
        in_maps.append(
            {
                "xt": xts[b],
                "wq": to_fp32r(Wq[:, sl]),
                "wk": to_fp32r(Wk[:, sl]),
                "wv": to_fp32r(Wv[:, sl]),
                "wo": to_fp32r(Wo[perm][:, sl]),
                "bq": np.ascontiguousarray(bq[sl].reshape(2, P).T),
                "bk": np.ascontiguousarray(bk[sl].reshape(2, P).T),
                "bv": bv[sl].reshape(1, FT).copy(),
                "bo": bo[sl].reshape(1, FT).copy(),
                "tri": tri,
            }
        )

    LAST_RESULTS = run_bass_kernel_spmd(
        _nc_cache, in_maps, list(range(8)), trace=TRACE
    )
    res = LAST_RESULTS.results
    outp = np.empty((B, S, D), np.float32)
    for c in range(8):
        b, g = divmod(c, 4)
        outp[b, :, g * FT : (g + 1) * FT] = res[c]["out"]
    return outp


# revision 20
# speedup vs baseline: 1.5702x; 1.3215x over previous
"""Causal self-attention (B=2, S=2048, D=1024, H=16) on 8 trn2 NeuronCores.

Sharding: core c = b*4 + g handles batch b and head group g (4 heads,
256 features).  Per core:
  - QT/KT computed transposed (feature on partitions, seq on free dim)
    so the QK^T matmul needs no on-chip transposes (x is transposed on
    host, shared by the 4 cores of the batch).
  - logits computed transposed (sk on partitions, sq free) in 1024-wide
    tiles: softmax sum over sk comes free from an extra ones-column in
    the V matmul lhsT; exp runs as one wide ACT op per sk-block.
  - attention operands (q/k/v/attn weights) are bf16 on the PE (fp32
    PSUM accumulation); projections and the output GEMM use fp32r
    (fp32 rounded to 11 mantissa bits, full-rate on the PE).
  - causal masking per 128x128 block: fully-masked column ranges are
    memset to zero (never exp'd); the diagonal block gets a triangular
    mask multiply.
  - ctx^T (per head, normalized) is AllGather'd per head over the
    4-core group (overlapping the next head's compute); each core then
    computes its 256-column strip of the output projection.
Host only slices/transposes/permutes/concatenates (no arithmetic).
"""

import numpy as np

import concourse.bass as bass
import concourse.mybir as mybir
import concourse.tile as tile
from concourse import bacc
from concourse.bass_utils import run_bass_kernel_spmd

P = 128
B, S, D, H, DH = 2, 2048, 1024, 16, 64
FT = 256          # features per core (4 heads)
NHL = 4           # heads per core
KO = D // P       # 8 k-tiles over the model dim
ST = S // 512     # 4 sq tiles of 512
SB = S // P       # 16 seq blocks of 128
F32 = mybir.dt.float32
F32R = mybir.dt.float32r
BF16 = mybir.dt.bfloat16

TRACE = False
USE_DIVIDE = False
LAST_RESULTS = None
_nc_cache = None

try:
    import ml_dtypes
    _BF16_NP = np.dtype(ml_dtypes.bfloat16)
except ImportError:  # pragma: no cover
    _BF16_NP = np.float32


def to_fp32r(a):
    """Round fp32 to the fp32r format (E8M11: low 12 mantissa bits zero),
    round-to-nearest-even, matching the hardware downconv."""
    u = np.ascontiguousarray(a, np.float32).view(np.uint32)
    rb = (u >> 12) & 1
    u = u + 0x7FF + rb
    u = u & np.uint32(0xFFFFF000)
    return u.view(np.float32)


def _build_program():
    nc = bacc.Bacc(None, target_bir_lowering=False, debug=False, num_devices=8)

    xT = nc.dram_tensor("xt", [D, S], F32R, kind="ExternalInput").ap()
    wq = nc.dram_tensor("wq", [D, FT], F32R, kind="ExternalInput").ap()
    wk = nc.dram_tensor("wk", [D, FT], F32R, kind="ExternalInput").ap()
    wv = nc.dram_tensor("wv", [D, FT], F32R, kind="ExternalInput").ap()
    wo = nc.dram_tensor("wo", [D, FT], BF16, kind="ExternalInput").ap()
    bq = nc.dram_tensor("bq", [P, 2], F32, kind="ExternalInput").ap()
    bk = nc.dram_tensor("bk", [P, 2], F32, kind="ExternalInput").ap()
    bv = nc.dram_tensor("bv", [1, FT], F32, kind="ExternalInput").ap()
    bo = nc.dram_tensor("bo", [1, FT], F32, kind="ExternalInput").ap()
    tri = nc.dram_tensor("tri", [P, P], BF16, kind="ExternalInput").ap()
    out = nc.dram_tensor("out", [S, FT], F32, kind="ExternalOutput").ap()

    with tile.TileContext(nc) as tc:
        with (
            tc.tile_pool(name="const", bufs=1) as constp,
            tc.tile_pool(name="big", bufs=1) as bigp,
            tc.tile_pool(name="qk", bufs=1) as qkp,
            tc.tile_pool(name="attn", bufs=6) as attnp,
            tc.tile_pool(name="small", bufs=6) as smallp,
            tc.tile_pool(name="outp", bufs=3) as outp,
            tc.tile_pool(name="ps_mm", bufs=2, space="PSUM") as ps_mm,
            tc.tile_pool(name="ps_l", bufs=2, space="PSUM") as ps_l,
            tc.tile_pool(name="ps_ctx", bufs=2, space="PSUM") as ps_ctx,
            tc.tile_pool(name="dram", bufs=1, space="DRAM") as dramp,
        ):
            # ---- constants into SBUF
            wq_sb = constp.tile([P, KO, FT], F32R, tag="wq")
            wk_sb = constp.tile([P, KO, FT], F32R, tag="wk")
            wv_sb = constp.tile([P, KO, FT], F32R, tag="wv")
            wo_sb = constp.tile([P, KO, FT], BF16, tag="wo")
            for wsb, wdr in ((wq_sb, wq), (wk_sb, wk), (wv_sb, wv), (wo_sb, wo)):
                nc.gpsimd.dma_start(wsb[:], wdr.rearrange("(ko p) n -> p ko n", p=P))
            bq_sb = constp.tile([P, 2], F32, tag="bq")
            bk_sb = constp.tile([P, 2], F32, tag="bk")
            nc.sync.dma_start(bq_sb[:], bq)
            nc.sync.dma_start(bk_sb[:], bk)
            bv_sb = constp.tile([P, FT], F32, tag="bvb")
            bo_sb = constp.tile([P, FT], F32, tag="bob")
            nc.sync.dma_start(bv_sb[:], bv.partition_broadcast(P))
            nc.sync.dma_start(bo_sb[:], bo.partition_broadcast(P))
            tri_sb = constp.tile([P, P], BF16, tag="tri")
            nc.sync.dma_start(tri_sb[:], tri)
            # f32 zero tile: source for writing constants into f32r/bf16 tiles
            z_sb = constp.tile([P, DH], F32, tag="z")
            nc.vector.memset(z_sb[:], 0.0)

            # ---- x^T into SBUF (8 MB) on the gpsimd queue, one DMA per k-tile
            xT_sb = bigp.tile([P, KO, S], F32R, tag="big")
            for ko in range(KO):
                for half in range(2):
                    nc.sync.dma_start(
                        xT_sb[:, ko, half * 1024 : (half + 1) * 1024],
                        xT[ko * P : (ko + 1) * P, half * 1024 : (half + 1) * 1024],
                    )

            # ---- Q^T, K^T projections (feature-major, bf16 out), Q scaled
            qt_sb = qkp.tile([P, 2, S], BF16, tag="qt")
            kt_sb = qkp.tile([P, 2, S], BF16, tag="kt")
            for wsb, bsb, scale, dest in (
                (wq_sb, bq_sb, 1.0 / np.sqrt(DH), qt_sb),
                (wk_sb, bk_sb, 1.0, kt_sb),
            ):
                for m in range(2):
                    for t in range(ST):
                        ps = ps_mm.tile([P, 512], F32, tag="psmm")
                        for ko in range(KO):
                            nc.tensor.matmul(
                                ps[:, :],
                                wsb[:, ko, m * P : (m + 1) * P],
                                xT_sb[:, ko, t * 512 : (t + 1) * 512],
                                start=(ko == 0),
                                stop=(ko == KO - 1),
                            )
                        nc.vector.tensor_scalar(
                            dest[:, m, t * 512 : (t + 1) * 512],
                            ps[:, :],
                            scale,
                            bsb[:, m : m + 1],
                            mybir.AluOpType.mult,
                            mybir.AluOpType.add,
                        )

            # ---- V (natural layout, bf16) with an appended ones column per head
            v_sb = qkp.tile([P, SB, NHL * (DH + 1)], BF16, tag="v")
            v_ones = v_sb[:].rearrange("p s (h c) -> p s h c", c=DH + 1)[:, :, :, DH]
            nc.vector.tensor_scalar(
                v_ones,
                z_sb[:, None, :NHL].to_broadcast((P, SB, NHL)),
                0.0,
                1.0,
                mybir.AluOpType.mult,
                mybir.AluOpType.add,
            )
            ones_sb = constp.tile([1, DH], F32, tag="ones")
            nc.vector.memset(ones_sb[:], 1.0)
            for s in range(SB):
                ps = ps_mm.tile([P, 512], F32, tag="psmm")
                for ko in range(KO):
                    nc.tensor.matmul(
                        ps[:, :FT],
                        xT_sb[:, ko, s * P : (s + 1) * P],
                        wv_sb[:, ko, :],
                        start=(ko == 0),
                        stop=(ko == KO - 1),
                    )
                for h in range(NHL):
                    nc.vector.tensor_add(
                        v_sb[:, s, h * 65 : h * 65 + DH],
                        ps[:, h * DH : (h + 1) * DH],
                        bv_sb[:, h * DH : (h + 1) * DH],
                    )

            # ---- attention: per head, 1024-wide sq tiles; AllGather per head
            ag_in = [
                dramp.tile([DH, S], BF16, name=f"ag_in{h}") for h in range(NHL)
            ]
            ag_out = [
                dramp.tile([4 * DH, S], BF16, name=f"ag_out{h}") for h in range(NHL)
            ]
            lh = bigp.tile([P, KO, S], BF16, tag="big")  # Wo lhsT, reuses xT slot

            for h in range(NHL):
                th, rb = h // 2, (h % 2) * DH
                for t2 in range(2):
                    sq0 = t2 * 1024
                    nblk = 8 * t2 + 8   # sk blocks 0 .. 8*t2+7
                    pctx = [
                        ps_ctx.tile([P, 512], F32, tag="pctx", name=f"pctx{h}_{t2}_{q}")
                        for q in range(2)
                    ]
                    for i in range(nblk):
                        v0 = i <= 8 * t2 + 3     # half0 (sq0..sq0+511) has live cols
                        pl = ps_l.tile([P, 1024], F32, tag="pl")
                        if v0:
                            nc.tensor.matmul(
                                pl[:, 0:512],
                                kt_sb[rb : rb + DH, th, i * P : (i + 1) * P],
                                qt_sb[rb : rb + DH, th, sq0 : sq0 + 512],
                                start=True,
                                stop=True,
                            )
                        nc.tensor.matmul(
                            pl[:, 512:1024],
                            kt_sb[rb : rb + DH, th, i * P : (i + 1) * P],
                            qt_sb[rb : rb + DH, th, sq0 + 512 : sq0 + 1024],
                            start=True,
                            stop=True,
                        )
                        at = attnp.tile([P, 1024], BF16, tag="at")
                        if v0:
                            c = i - 8 * t2       # diag block index within half0
                            if c >= 0:
                                if c > 0:
                                    nc.vector.memset(at[:, : c * P], 0.0)
                                nc.scalar.activation(
                                    at[:, c * P :],
                                    pl[:, c * P :],
                                    mybir.ActivationFunctionType.Exp,
                                )
                                nc.vector.tensor_mul(
                                    at[:, c * P : (c + 1) * P],
                                    at[:, c * P : (c + 1) * P],
                                    tri_sb[:, :],
                                )
                            else:
                                nc.scalar.activation(
                                    at[:, :], pl[:, :],
                                    mybir.ActivationFunctionType.Exp,
                                )
                        else:
                            c = i - (8 * t2 + 4)  # diag block within half1
                            base = 512 + c * P
                            if c > 0:
                                nc.vector.memset(at[:, 512 : base], 0.0)
                            nc.scalar.activation(
                                at[:, base:],
                                pl[:, base:],
                                mybir.ActivationFunctionType.Exp,
                            )
                            nc.vector.tensor_mul(
                                at[:, base : base + P],
                                at[:, base : base + P],
                                tri_sb[:, :],
                            )
                        if v0:
                            nc.tensor.matmul(
                                pctx[0][0 : DH + 1, :],
                                v_sb[:, i, h * 65 : (h + 1) * 65],
                                at[:, 0:512],
                                start=(i == 0),
                                stop=(i == 8 * t2 + 3),
                            )
                        nc.tensor.matmul(
                            pctx[1][0 : DH + 1, :],
                            v_sb[:, i, h * 65 : (h + 1) * 65],
                            at[:, 512:1024],
                            start=(i == 0),
                            stop=(i == nblk - 1),
                        )
                    # normalize the two finished 512-quarters
                    for hh in range(2):
                        tq = 2 * t2 + hh
                        cn = smallp.tile([DH, 512], BF16, tag="cn")
                        rsc = smallp.tile([1, 512], F32, tag="rsc")
                        nc.vector.tensor_copy(rsc[:], pctx[hh][DH : DH + 1, :])
                        rs = smallp.tile([1, 512], F32, tag="rs")
                        nc.vector.reciprocal_approx_fast(rs[:], rsc[:])
                        pbc = ps_mm.tile([P, 512], F32, tag="psmm")
                        nc.tensor.matmul(
                            pbc[0:DH, :], ones_sb[:, :], rs[:, :],
                            start=True, stop=True,
                        )
                        bc = smallp.tile([DH, 512], F32, tag="bc")
                        nc.vector.tensor_copy(bc[:], pbc[0:DH, :])
                        nc.vector.tensor_mul(cn[:], pctx[hh][0:DH, :], bc[:])
                        nc.sync.dma_start(
                            ag_in[h][:, tq * 512 : (tq + 1) * 512], cn[:]
                        )
                # per-head AllGather (overlaps the next head's compute)
                nc.gpsimd.collective_compute(
                    "AllGather",
                    mybir.AluOpType.bypass,
                    replica_groups=[[0, 1, 2, 3], [4, 5, 6, 7]],
                    ins=[ag_in[h].opt()],
                    outs=[ag_out[h].opt()],
                )

            # gathered ctx^T k-tiles -> SBUF (issued after attention so the
            # AG-completion waits don't block any queue mid-attention)
            for h in range(NHL):
                for kk in range(2):
                    nc.sync.dma_start(
                        lh[:, 2 * h + kk, :],
                        ag_out[h][kk * P : (kk + 1) * P, :],
                    )

            # ---- output projection: out[:, my 256 cols] = ctx @ Wo_perm + bo
            for m in range(SB):
                ps = ps_mm.tile([P, 512], F32, tag="psmm")
                for ko in range(KO):
                    nc.tensor.matmul(
                        ps[:, :FT],
                        lh[:, ko, m * P : (m + 1) * P],
                        wo_sb[:, ko, :],
                        start=(ko == 0),
                        stop=(ko == KO - 1),
                    )
                ob = outp.tile([P, FT], F32, tag="ob")
                nc.vector.tensor_add(ob[:], ps[:, :FT], bo_sb[:])
                nc.sync.dma_start(out[m * P : (m + 1) * P, :], ob[:])

    nc.finalize()
    return nc


def kernel(x, Wq, bq, Wk, bk, Wv, bv, Wo, bo):
    global _nc_cache, LAST_RESULTS
    if _nc_cache is None:
        _nc_cache = _build_program()

    x = np.asarray(x, np.float32)
    Wq, Wk, Wv, Wo = (np.asarray(a, np.float32) for a in (Wq, Wk, Wv, Wo))
    bq, bk, bv, bo = (np.asarray(a, np.float32) for a in (bq, bk, bv, bo))
    tri = np.triu(np.ones((P, P), np.float32)).astype(_BF16_NP)

    # Wo row permutation: the gathered ctx^T k-tiles arrive ordered
    # (head h, group g, row r) -> global feature g*256 + h*64 + r
    perm = np.array(
        [
            j * FT + h * DH + r
            for h in range(NHL)
            for j in range(4)
            for r in range(DH)
        ]
    )

    xts = [to_fp32r(np.ascontiguousarray(x[b].T)) for b in range(B)]
    in_maps = []
    for c in range(8):
        b, g = divmod(c, 4)
        sl = slice(g * FT, (g + 1) * FT)
        in_maps.append(
            {
                "xt": xts[b],
                "wq": to_fp32r(Wq[:, sl]),
                "wk": to_fp32r(Wk[:, sl]),
                "wv": to_fp32r(Wv[:, sl]),
                "wo": Wo[perm][:, sl].astype(_BF16_NP),
                "bq": np.ascontiguousarray(bq[sl].reshape(2, P).T),
                "bk": np.ascontiguousarray(bk[sl].reshape(2, P).T),
                "bv": bv[sl].reshape(1, FT).copy(),
                "bo": bo[sl].reshape(1, FT).copy(),
                "tri": tri,
            }
        )

    LAST_RESULTS = run_bass_kernel_spmd(
        _nc_cache, in_maps, list(range(8)), trace=TRACE
    )
    res = LAST_RESULTS.results
    outp = np.empty((B, S, D), np.float32)
    for c in range(8):
        b, g = divmod(c, 4)
        outp[b, :, g * FT : (g + 1) * FT] = res[c]["out"]
    return outp
